# revision 12
# baseline (speedup 1.0000x reference)
"""CMAA layer (4-node complete-graph gated cross-attention + LN) on 8 trn2 cores.

Sharding: pure data-parallel over batch B=32768 -> 8 x 4096 graphs.
Per-core layout strategy:
  - tokens passed per-node, both natural [b, d] and transposed [d, b]
  - Q/K projections weight-stationary -> Q^T/K^T [d, b] (transposed)
  - V/R projections token-stationary  -> V|R natural [b, d]
  - scores: DVE pair-products in transposed layout + PE head-mask matmul reduce
  - softmax: compact [b, 96] natural after PE transpose of score rows
  - AV: broadcast-AP TT mul-adds in natural layout
  - Wo: token-stationary (lhsT = AO^T from PE transpose), residual add + LN natural
"""

import sys

if "/opt/trn_rl_repo" not in sys.path:
    sys.path.insert(0, "/opt/trn_rl_repo")

import numpy as np
from contextlib import ExitStack

import concourse.bacc as bacc
import concourse.bass as bass
import concourse.tile as tile
import concourse.mybir as mybir
from concourse import bass2jax

F32 = mybir.dt.float32
F32R = mybir.dt.float32r
BF16 = mybir.dt.bfloat16

B, D, H = 32768, 256, 8
DH = D // H
SCALE = 1.0 / float(np.sqrt(DH))
NCORES = 8
BC = B // NCORES  # graphs per core

# l(i, r): position of node j=(i+r)%4 in NEIGH[i] (sorted others)
NEIGH = [[1, 2, 3], [0, 2, 3], [0, 1, 3], [0, 1, 2]]
L_OF = {(i, r): NEIGH[i].index((i + r) % 4) for i in range(4) for r in (1, 2, 3)}


def build_kernel(bc=BC, nb=512):
    """Build the per-core Bass program. bc = graphs per core, nb = graphs per chunk."""
    assert bc % nb == 0 and nb % 128 == 0
    nchunk = bc // nb
    nt = nb // 128  # 128-row subtiles per chunk

    nc = bacc.Bacc("TRN2", target_bir_lowering=False, debug=False)

    tt = nc.dram_tensor("tt", [4, D, bc], F32R, kind="ExternalInput").ap()
    tn = nc.dram_tensor("tn", [4, bc, D], F32, kind="ExternalInput").ap()
    wq_t = nc.dram_tensor("wq_t", [D, D], F32R, kind="ExternalInput").ap()
    wk_t = nc.dram_tensor("wk_t", [D, D], F32R, kind="ExternalInput").ap()
    wvr = nc.dram_tensor("wvr", [D, 2 * D], F32R, kind="ExternalInput").ap()
    wo_t = nc.dram_tensor("wo_t", [D, D], F32R, kind="ExternalInput").ap()
    gb = nc.dram_tensor("gb", [2, 128, D], F32, kind="ExternalInput").ap()
    # hmask[kk, pair, d_row, col]: mask matmul lhsT per d-chunk kk and (i,r) pair;
    # col (i*24 + l*8 + h) is 1.0 on head-h rows of chunk kk for this pair only.
    hmask = nc.dram_tensor("hmask", [2, 12, 128, 96], F32R, kind="ExternalInput").ap()
    ident = nc.dram_tensor("ident", [128, 128], F32, kind="ExternalInput").ap()
    y_out = nc.dram_tensor("y", [4, bc, D], F32, kind="ExternalOutput").ap()
    attn_out = nc.dram_tensor("attn", [bc, 96], F32, kind="ExternalOutput").ap()

    def r32(ap):
        return ap.bitcast(F32R)

    with tile.TileContext(nc) as tc, ExitStack() as ctx:
        # ---- persistent constants ----
        wpool = ctx.enter_context(tc.tile_pool(name="w", bufs=1))
        wq_sb = wpool.tile([128, 2, D], F32R, tag="wq")
        nc.sync.dma_start(wq_sb[:], wq_t.rearrange("(k p) m -> p k m", p=128))
        wk_sb = wpool.tile([128, 2, D], F32R, tag="wk")
        nc.sync.dma_start(wk_sb[:], wk_t.rearrange("(k p) m -> p k m", p=128))
        wvr_sb = wpool.tile([128, 2, 2 * D], F32R, tag="wvr")
        nc.sync.dma_start(wvr_sb[:], wvr.rearrange("(k p) m -> p k m", p=128))
        wo_sb = wpool.tile([128, 2, D], F32R, tag="wo")
        nc.sync.dma_start(wo_sb[:], wo_t.rearrange("(k p) m -> p k m", p=128))
        g_sb = wpool.tile([128, D], F32, tag="g")
        nc.sync.dma_start(g_sb[:], gb[0])
        b_sb = wpool.tile([128, D], F32, tag="b")
        nc.sync.dma_start(b_sb[:], gb[1])
        hm_sb = wpool.tile([128, 2, 12, 96], F32R, tag="hm")
        nc.sync.dma_start(hm_sb[:], hmask.rearrange("k q p m -> p k q m"))
        id_sb = wpool.tile([128, 128], F32, tag="id")
        nc.sync.dma_start(id_sb[:], ident)

        # ---- per-chunk pools ----
        ttq_pool = ctx.enter_context(tc.tile_pool(name="ttq", bufs=2))
        tn_pool = ctx.enter_context(tc.tile_pool(name="tn", bufs=2))
        qkT_pool = ctx.enter_context(tc.tile_pool(name="qkT", bufs=1))
        veff_pool = ctx.enter_context(tc.tile_pool(name="veff", bufs=1))
        sig_pool = ctx.enter_context(tc.tile_pool(name="sig", bufs=2))
        p_pool = ctx.enter_context(tc.tile_pool(name="pprod", bufs=4))
        cmpk_pool = ctx.enter_context(tc.tile_pool(name="cmpk", bufs=2))
        ao_pool = ctx.enter_context(tc.tile_pool(name="ao", bufs=1))
        aoT_pool = ctx.enter_context(tc.tile_pool(name="aoT", bufs=2))
        y_pool = ctx.enter_context(tc.tile_pool(name="ysb", bufs=2))
        st_pool = ctx.enter_context(tc.tile_pool(name="stats", bufs=2))

        qk_ps = ctx.enter_context(tc.tile_pool(name="qk_ps", bufs=2, space="PSUM"))
        vr_ps = ctx.enter_context(tc.tile_pool(name="vr_ps", bufs=2, space="PSUM"))
        s_ps = ctx.enter_context(tc.tile_pool(name="s_ps", bufs=1, space="PSUM"))
        sn_ps = ctx.enter_context(tc.tile_pool(name="sn_ps", bufs=1, space="PSUM"))
        aoT_ps = ctx.enter_context(tc.tile_pool(name="aoT_ps", bufs=1, space="PSUM"))
        y_ps = ctx.enter_context(tc.tile_pool(name="y_ps", bufs=1, space="PSUM"))

        for cidx in range(nchunk):
            c0 = cidx * nb

            # ---- load transposed tokens for this chunk ----
            ttq = []
            for i in range(4):
                t = ttq_pool.tile([128, 2, nb], F32R, tag=f"ttq{i}")
                nc.sync.dma_start(
                    t[:], tt[i, :, c0 : c0 + nb].rearrange("(k p) n -> p k n", p=128)
                )
                ttq.append(t)

            # ---- A: Q/K projections (weight-stationary, out transposed) ----
            qT, kT = [], []
            for i in range(4):
                for w_sb, dst_list, tag in ((wq_sb, qT, "q"), (wk_sb, kT, "k")):
                    dst = qkT_pool.tile([128, 2, nb], F32, tag=f"{tag}T{i}")
                    for m in range(2):
                        ps = qk_ps.tile([128, nb], F32, tag="qk")
                        for kk in range(2):
                            nc.tensor.matmul(
                                ps[:],
                                w_sb[:, kk, m * 128 : (m + 1) * 128],
                                ttq[i][:, kk, :],
                                start=(kk == 0),
                                stop=(kk == 1),
                            )
                        if tag == "q":
                            nc.scalar.copy(dst[:, m, :], ps[:])
                        else:
                            nc.vector.tensor_copy(dst[:, m, :], ps[:])
                    dst_list.append(dst)

            # ---- B: V/R projections (token-stationary, out natural) + gate ----
            veff = []
            for i in range(4):
                ve = veff_pool.tile([128, nt, D], F32, tag=f"veff{i}")
                for t in range(nt):
                    ps = vr_ps.tile([128, 2 * D], F32, tag="vr")
                    for kk in range(2):
                        nc.tensor.matmul(
                            ps[:],
                            ttq[i][:, kk, t * 128 : (t + 1) * 128],
                            wvr_sb[:, kk, :],
                            start=(kk == 0),
                            stop=(kk == 1),
                        )
                    sg = sig_pool.tile([128, D], F32, tag="sig")
                    nc.scalar.activation(
                        sg[:], ps[:, D:], mybir.ActivationFunctionType.Sigmoid
                    )
                    nc.vector.tensor_mul(ve[:, t, :], sg[:], ps[:, :D])
                veff.append(ve)

            # ---- C: scores -> s^T rows (i, l, h) in PSUM ----
            # All 24 mask-matmuls accumulate into one [96, nb] PSUM region
            # (masks place each pair's 8 head-columns; matmul out must start
            # at partition 0, so placement lives in the mask columns).
            sT = s_ps.tile([128, nb], F32, tag="sT")
            pairs = [(i, r) for i in range(4) for r in (1, 2, 3)]
            n_mm = 0
            for pidx, (i, r) in enumerate(pairs):
                j = (i + r) % 4
                for kk in range(2):
                    pp = p_pool.tile([128, nb], F32R, tag="pp")
                    nc.vector.tensor_mul(pp[:], qT[i][:, kk, :], kT[j][:, kk, :])
                    nc.tensor.matmul(
                        sT[0:96, :],
                        hm_sb[:, kk, pidx, :],
                        pp[:],
                        start=(n_mm == 0),
                        stop=(n_mm == 23),
                    )
                    n_mm += 1

            # evac s^T -> SBUF (zero the pad rows once; transpose reads them)
            sT_sb = cmpk_pool.tile([128, nb], F32, tag="sT_sb")
            nc.vector.memset(sT_sb[96:128, :], 0.0)
            nc.scalar.copy(sT_sb[0:96, :], sT[0:96, :])

            # ---- D/E: transpose to natural compact, softmax, attn out ----
            a_nat = []
            for t in range(nt):
                sn = sn_ps.tile([128, 128], F32, tag="sn")
                nc.tensor.transpose(
                    sn[:], sT_sb[:, t * 128 : (t + 1) * 128], id_sb[:]
                )
                e_sb = cmpk_pool.tile([128, 96], F32, tag="e")
                nc.scalar.activation(
                    e_sb[:], sn[:, 0:96], mybir.ActivationFunctionType.Exp,
                    scale=SCALE,
                )
                # reduce over l (stride 8), keeping (i [s24], h [s1])
                z_sb = cmpk_pool.tile([128, 32], F32, tag="z")
                e_ilh = e_sb[:].rearrange("p (i l h) -> p i h l", i=4, l=3)
                nc.vector.reduce_sum(
                    z_sb[:].rearrange("p (i h) -> p i h", i=4),
                    e_ilh,
                    axis=mybir.AxisListType.X,
                )
                rz_sb = cmpk_pool.tile([128, 32], F32, tag="rz")
                nc.vector.reciprocal(rz_sb[:], z_sb[:])
                # a[(i,h,l)] = E[(i,l,h)] * rz[(i,h)], emitted in attn layout
                a_sb = cmpk_pool.tile([128, 96], F32, tag="a")
                a_view = a_sb[:].rearrange("p (i h l) -> p i l h", i=4, h=8)
                e_view = e_sb[:].rearrange("p (i l h) -> p i l h", i=4, l=3)
                rz_view = rz_sb[:].rearrange("p (i h) -> p i h", i=4).unsqueeze(2)
                rz_bc = rz_view.broadcast_to((128, 4, 3, 8))
                nc.vector.tensor_mul(a_view, e_view, rz_bc)
                nc.sync.dma_start(
                    attn_out[c0 + t * 128 : c0 + (t + 1) * 128, :], a_sb[:]
                )
                a_nat.append(a_sb)

            # ---- F: AV in natural layout (t-outer so a_nat[t] releases early) ----
            ao = []
            for i in range(4):
                av_i = ao_pool.tile([128, nt, D], F32, tag=f"ao{i}")
                ao.append(av_i)
            for t in range(nt):
                for i in range(4):
                    av = ao[i]
                    first = True
                    for r in (1, 2, 3):
                        j = (i + r) % 4
                        l = L_OF[(i, r)]
                        a_ihl = a_nat[t][:].rearrange(
                            "p (i h l) -> p i h l", i=4, h=8
                        )
                        a_bc = (
                            a_ihl[:, i, :, l].unsqueeze(2).broadcast_to((128, 8, DH))
                        )
                        v_view = veff[j][:, t, :].rearrange(
                            "p (h e) -> p h e", h=8
                        )
                        if first:
                            nc.vector.tensor_mul(
                                av[:, t, :].rearrange("p (h e) -> p h e", h=8),
                                v_view,
                                a_bc,
                            )
                            first = False
                        else:
                            tmp = p_pool.tile([128, D], F32, tag="avtmp")
                            nc.vector.tensor_mul(
                                tmp[:].rearrange("p (h e) -> p h e", h=8),
                                v_view,
                                a_bc,
                            )
                            nc.vector.tensor_add(av[:, t, :], av[:, t, :], tmp[:])

            # ---- G: AO transpose + Wo (token-stationary) + residual ----
            for i in range(4):
                aoT_sb = aoT_pool.tile([128, 2, nb], F32R, tag="aoT")
                for m in range(2):
                    aops = aoT_ps.tile([128, nb], F32, tag="aoT_ps")
                    for t in range(nt):
                        nc.tensor.transpose(
                            aops[:, t * 128 : (t + 1) * 128],
                            ao[i][:, t, m * 128 : (m + 1) * 128],
                            id_sb[:],
                        )
                    nc.vector.tensor_copy(aoT_sb[:, m, :], aops[:])

                tn_sb = tn_pool.tile([128, nt, D], F32, tag="tn")
                nc.sync.dma_start(
                    tn_sb[:],
                    tn[i, c0 : c0 + nb, :].rearrange("(t p) d -> p t d", p=128),
                )
                for t in range(nt):
                    yps = y_ps.tile([128, D], F32, tag="y_ps")
                    for kk in range(2):
                        nc.tensor.matmul(
                            yps[:],
                            aoT_sb[:, kk, t * 128 : (t + 1) * 128],
                            wo_sb[:, kk, :],
                            start=(kk == 0),
                            stop=(kk == 1),
                        )
                    y_sb = y_pool.tile([128, D], F32, tag="y_sb")
                    nc.vector.tensor_add(y_sb[:], yps[:], tn_sb[:, t, :])

                    # ---- H: LayerNorm ----
                    mu = st_pool.tile([128, 1], F32, tag="mu")
                    nc.vector.reduce_sum(mu[:], y_sb[:], axis=mybir.AxisListType.X)
                    mu_s = st_pool.tile([128, 1], F32, tag="mu_s")
                    nc.scalar.mul(mu_s[:], mu[:], 1.0 / D)
                    ysq = st_pool.tile([128, D], F32, tag="ysq")
                    ssq = st_pool.tile([128, 1], F32, tag="ssq")
                    nc.scalar.activation(
                        ysq[:], y_sb[:], mybir.ActivationFunctionType.Square,
                        accum_out=ssq[:],
                    )
                    # var = ssq/D - mu_s^2 ; rstd = 1/sqrt(var+eps)
                    mu2 = st_pool.tile([128, 1], F32, tag="mu2")
                    nc.scalar.square(mu2[:], mu_s[:])
                    var = st_pool.tile([128, 1], F32, tag="var")
                    nc.vector.scalar_tensor_tensor(
                        var[:], ssq[:], 1.0 / D, mu2[:],
                        op0=mybir.AluOpType.mult, op1=mybir.AluOpType.subtract,
                    )
                    vare = st_pool.tile([128, 1], F32, tag="vare")
                    nc.vector.tensor_scalar_add(vare[:], var[:], 1e-5)
                    sq = st_pool.tile([128, 1], F32, tag="sq")
                    nc.scalar.activation(
                        sq[:], vare[:], mybir.ActivationFunctionType.Sqrt,
                    )
                    rstd = st_pool.tile([128, 1], F32, tag="rstd")
                    nc.vector.reciprocal(rstd[:], sq[:])
                    t1 = st_pool.tile([128, D], F32, tag="t1")
                    nc.vector.scalar_tensor_tensor(
                        t1[:], y_sb[:], mu_s[:], g_sb[:],
                        op0=mybir.AluOpType.subtract, op1=mybir.AluOpType.mult,
                    )
                    yo = y_pool.tile([128, D], F32, tag="yo")
                    nc.vector.scalar_tensor_tensor(
                        yo[:], t1[:], rstd[:], b_sb[:],
                        op0=mybir.AluOpType.mult, op1=mybir.AluOpType.add,
                    )
                    nc.sync.dma_start(
                        y_out[i, c0 + t * 128 : c0 + (t + 1) * 128, :], yo[:]
                    )

    nc.compile()
    return nc


def make_inputs(tokens, Wq, Wk, Wv, Wr, Wo, ln_g, ln_b, bc=BC):
    """Host-side prep: shard + layout transforms. Returns list of per-core input maps."""
    tokens = np.asarray(tokens, dtype=np.float32)
    n = tokens.shape[0] // bc
    hm = np.zeros((2, 12, 128, 96), dtype=np.float32)
    pairs = [(i, r) for i in range(4) for r in (1, 2, 3)]
    for pidx, (i, r) in enumerate(pairs):
        l = L_OF[(i, r)]
        for h in range(H):
            kk, off = divmod(h * DH, 128)
            hm[kk, pidx, off : off + DH, i * 24 + l * 8 + h] = 1.0
    const = {
        "wq_t": np.ascontiguousarray(np.asarray(Wq, np.float32).T),
        "wk_t": np.ascontiguousarray(np.asarray(Wk, np.float32).T),
        "wvr": np.ascontiguousarray(
            np.concatenate([np.asarray(Wv, np.float32).T, np.asarray(Wr, np.float32).T], axis=1)
        ),
        "wo_t": np.ascontiguousarray(np.asarray(Wo, np.float32).T),
        "gb": np.stack(
            [
                np.broadcast_to(np.asarray(ln_g, np.float32), (128, D)),
                np.broadcast_to(np.asarray(ln_b, np.float32), (128, D)),
            ]
        ).copy(),
        "hmask": hm,
        "ident": np.eye(128, dtype=np.float32),
    }
    maps = []
    for c in range(n):
        sh = tokens[c * bc : (c + 1) * bc]  # [bc, 4, D]
        tn_ = np.ascontiguousarray(sh.transpose(1, 0, 2))  # [4, bc, D]
        tt_ = np.ascontiguousarray(sh.transpose(1, 2, 0))  # [4, D, bc]
        maps.append({"tt": tt_, "tn": tn_, **const})
    return maps


def assemble_outputs(results, bc=BC):
    """results: list of per-core dicts with 'y' [4, bc, D] and 'attn' [bc, 96]."""
    n = len(results)
    y = np.empty((n * bc, 4, D), dtype=np.float32)
    attn = np.empty((n * bc, 4, H, 3), dtype=np.float32)
    for c, res in enumerate(results):
        y[c * bc : (c + 1) * bc] = res["y"].transpose(1, 0, 2)
        attn[c * bc : (c + 1) * bc] = res["attn"].reshape(bc, 4, H, 3)
    return y, attn


_CACHE = {}


def _get_fn():
    if "fn" in _CACHE:
        return _CACHE["fn"]
    import jax
    from jax.sharding import Mesh, PartitionSpec
    from jax.experimental.shard_map import shard_map

    nc = build_kernel()
    bass2jax.install_neuronx_cc_hook()
    in_names, out_names, out_avals = [], [], []
    for alloc in nc.m.functions[0].allocations:
        if not isinstance(alloc, mybir.MemoryLocationSet):
            continue
        name = alloc.memorylocations[0].name
        if alloc.kind == "ExternalInput":
            if name != "partition_id":
                in_names.append(name)
        elif alloc.kind == "ExternalOutput":
            out_names.append(name)
            out_avals.append(
                __import__("jax").core.ShapedArray(
                    tuple(alloc.tensor_shape), mybir.dt.np(alloc.dtype)
                )
            )
    all_names = in_names + out_names + (
        ["partition_id"] if nc.partition_id_tensor is not None else []
    )

    def _body(*args):
        operands = list(args)
        if nc.partition_id_tensor is not None:
            operands.append(bass2jax.partition_id_tensor())
        outs = bass2jax._bass_exec_p.bind(
            *operands,
            out_avals=tuple(out_avals),
            in_names=tuple(all_names),
            out_names=tuple(out_names),
            lowering_input_output_aliases=(),
            sim_require_finite=True,
            sim_require_nnan=True,
            nc=nc,
        )
        return tuple(outs)

    devices = jax.devices()[:NCORES]
    mesh = Mesh(np.asarray(devices), ("core",))
    n_io = len(in_names) + len(out_names)
    fn = jax.jit(
        shard_map(
            _body,
            mesh=mesh,
            in_specs=(PartitionSpec("core"),) * n_io,
            out_specs=(PartitionSpec("core"),) * len(out_names),
            check_rep=False,
        ),
        keep_unused=True,
    )
    _CACHE["fn"] = (fn, in_names, out_names, out_avals)
    return _CACHE["fn"]


def kernel(tokens, Wq, Wk, Wv, Wr, Wo, ln_g, ln_b):
    fn, in_names, out_names, out_avals = _get_fn()
    maps = make_inputs(tokens, Wq, Wk, Wv, Wr, Wo, ln_g, ln_b)
    concat_in = [
        np.concatenate([maps[c][nm] for c in range(NCORES)], axis=0)
        for nm in in_names
    ]
    zeros = [
        np.zeros((NCORES * av.shape[0], *av.shape[1:]), av.dtype) for av in out_avals
    ]
    outs = fn(*concat_in, *zeros)
    results = [
        {
            nm: np.asarray(outs[k]).reshape(NCORES, *out_avals[k].shape)[c]
            for k, nm in enumerate(out_names)
        }
        for c in range(NCORES)
    ]
    return assemble_outputs(results)
